# Initial kernel scaffold
#
"""Cosine-similarity matrix kernel for 8 Trainium2 NeuronCores.

Computes sim = (x_norm @ y_norm.T) / TEMP for x, y: [4096, 1024] fp32.

Strategy:
  - 4x2 grid over the output: core c = (r, cg) computes the slab
    out[r*1024:(r+1)*1024, cg*2048:(cg+1)*2048].
  - Host pre-transposes x and y so the device receives [K, M]/[K, N]
    layouts (K on partitions) and a natural-layout copy of the x shard
    for exact row-norm computation on the Scalar engine.
  - Device: raw GEMM in float32r (full-rate PE) accumulating fp32 in
    PSUM; row/col norm scales are applied during PSUM evacuation with a
    single fused DVE scalar_tensor_tensor per output tile.
  - y row-norms via a ones-vector matmul over squared tiles (partition
    reduction on the PE); x row-norms via ACT Square+accumulate on the
    natural-layout shard.
"""

import sys

sys.path.insert(0, "/opt/trn_rl_repo")

import numpy as np

TEMP = 0.05
EPS = 1e-8

N_FULL = 4096  # rows of x (output rows)
M_FULL = 4096  # rows of y (output cols)
K = 1024       # feature dim (contraction)
GRID_R = 4     # x-row shards
GRID_C = 2     # y-row shards
ML = N_FULL // GRID_R   # 1024 output rows per core
NL = M_FULL // GRID_C   # 2048 output cols per core
KT = K // 128           # 8 contraction chunks
MT = ML // 128          # 8 m-tiles per core
NS = 512                # matmul moving free dim
NH = 2                  # n-slices per psum group
NG = NL // (NS * NH)    # 2 psum groups per m-tile

_CACHE = {}


def _build():
    import concourse.bacc as bacc
    import concourse.mybir as mybir
    import concourse.tile as tile

    f32 = mybir.dt.float32
    f32r = mybir.dt.float32r
    mult = mybir.AluOpType.mult

    nc = bacc.Bacc("TRN2")
    xt_d = nc.dram_tensor("xt", [K, ML], f32, kind="ExternalInput")   # x-shard transposed [K, ML]
    xn_d = nc.dram_tensor("xn", [ML, K], f32, kind="ExternalInput")   # x-shard natural [ML, K]
    yt_d = nc.dram_tensor("yt", [K, NL], f32, kind="ExternalInput")   # y-shard transposed [K, NL]
    out_d = nc.dram_tensor("out", [ML, NL], f32, kind="ExternalOutput")

    def r32(ap):
        return ap.bitcast(f32r)

    with tile.TileContext(nc) as tc:
        with (
            tc.tile_pool(name="big", bufs=1) as big,
            tc.tile_pool(name="sml", bufs=1) as sml,
            tc.tile_pool(name="scr", bufs=2) as scr,
            tc.tile_pool(name="ost", bufs=4) as ost,
            tc.tile_pool(name="psn", bufs=1, space="PSUM") as psn,
            tc.tile_pool(name="psg", bufs=2, space="PSUM") as psg,
        ):
            # ---- persistent SBUF inputs ----
            xt_sb = []
            yt_sb = []
            for k in range(KT):
                xt_k = big.tile([128, ML], f32, name=f"xt{k}", tag=f"xt{k}")
                nc.sync.dma_start(xt_k[:], xt_d[k * 128:(k + 1) * 128, :])
                xt_sb.append(xt_k)
                yt_k = big.tile([128, NL], f32, name=f"yt{k}", tag=f"yt{k}")
                nc.sync.dma_start(yt_k[:], yt_d[k * 128:(k + 1) * 128, :])
                yt_sb.append(yt_k)

            ones = sml.tile([128, 1], f32, name="ones", tag="ones")
            nc.vector.memset(ones, 1.0)

            # ---- x row norms from the natural-layout shard ----
            # rx[t][p] = 1 / (TEMP * max(||x_{t*128+p}||, EPS))
            rx = []
            for t in range(MT):
                xnat = scr.tile([128, K], f32, name=f"xnat{t}", tag="xnat")
                nc.sync.dma_start(xnat[:], xn_d[t * 128:(t + 1) * 128, :])
                sqd = scr.tile([128, K], f32, name=f"sqd{t}", tag="sqd")
                nx2 = sml.tile([128, 1], f32, name=f"nx2_{t}", tag=f"nx2_{t}")
                nc.scalar.activation(
                    sqd[:], xnat[:], mybir.ActivationFunctionType.Square,
                    accum_out=nx2[:],
                )
                rx_t = sml.tile([128, 1], f32, name=f"rx{t}", tag=f"rx{t}")
                nc.scalar.sqrt(rx_t[:], nx2[:])
                nc.vector.tensor_scalar_max(rx_t[:], rx_t[:], EPS)
                nc.vector.tensor_scalar_mul(rx_t[:], rx_t[:], TEMP)
                nc.vector.reciprocal(rx_t[:], rx_t[:])
                rx.append(rx_t)

            # ---- y row norms: ones-vector matmul over squared tiles ----
            # ny2[n] = sum_k yt[k, n]^2  (partition reduction on PE, f32r)
            ny_ps = psn.tile([1, NL // NS, NS], f32, name="ny_ps", tag="ny_ps")
            for k in range(KT):
                ysq = scr.tile([128, NL], f32, name=f"ysq{k}", tag="ysq")
                nc.scalar.square(ysq[:], yt_sb[k][:])
                for s in range(NL // NS):
                    nc.tensor.matmul(
                        ny_ps[:, s],
                        r32(ones[:]),
                        r32(ysq[:, s * NS:(s + 1) * NS]),
                        start=(k == 0),
                        stop=(k == KT - 1),
                    )
            ry_row = sml.tile([1, NL], f32, name="ry_row", tag="ry_row")
            for s in range(NL // NS):
                nc.vector.tensor_copy(ry_row[:, s * NS:(s + 1) * NS], ny_ps[:, s])
            nc.scalar.sqrt(ry_row[:], ry_row[:])
            nc.vector.tensor_scalar_max(ry_row[:], ry_row[:], EPS)
            nc.vector.reciprocal(ry_row[:], ry_row[:])
            # replicate across all 128 partitions for the DVE evac multiply
            ry_rep = big.tile([128, NL], f32, name="ry_rep", tag="ry_rep")
            nc.sync.dma_start(ry_rep[:], ry_row[0:1, :].partition_broadcast(128))

            # ---- main GEMM: out[m, n] = sum_k xt[k, m] * yt[k, n] ----
            for t in range(MT):
                lhsT = lambda k: r32(xt_sb[k][:, t * 128:(t + 1) * 128])
                for g in range(NG):
                    po = psg.tile([128, NH, NS], f32, name=f"po{t}_{g}", tag="po")
                    for k in range(KT):
                        lh = lhsT(k)
                        for s in range(NH):
                            off = (g * NH + s) * NS
                            nc.tensor.matmul(
                                po[:, s],
                                lh,
                                r32(yt_sb[k][:, off:off + NS]),
                                start=(k == 0),
                                stop=(k == KT - 1),
                            )
                    for s in range(NH):
                        off = (g * NH + s) * NS
                        osb = ost.tile([128, NS], f32, name=f"osb{t}_{g}_{s}", tag="osb")
                        nc.vector.scalar_tensor_tensor(
                            osb[:], po[:, s], rx[t][:], ry_rep[:, off:off + NS],
                            op0=mult, op1=mult,
                        )
                        nc.sync.dma_start(
                            out_d[t * 128:(t + 1) * 128, off:off + NS], osb[:]
                        )

    nc.finalize()
    return nc


def _get_nc():
    if "nc" not in _CACHE:
        _CACHE["nc"] = _build()
    return _CACHE["nc"]


def _shard(x, y):
    x = np.ascontiguousarray(np.asarray(x, dtype=np.float32))
    y = np.ascontiguousarray(np.asarray(y, dtype=np.float32))
    xT = np.ascontiguousarray(x.T)
    yT = np.ascontiguousarray(y.T)
    in_maps = []
    for c in range(8):
        r, cg = divmod(c, GRID_C)
        in_maps.append({
            "xt": np.ascontiguousarray(xT[:, r * ML:(r + 1) * ML]),
            "xn": np.ascontiguousarray(x[r * ML:(r + 1) * ML, :]),
            "yt": np.ascontiguousarray(yT[:, cg * NL:(cg + 1) * NL]),
        })
    return in_maps


def _gather(results):
    out = np.empty((N_FULL, M_FULL), np.float32)
    for c in range(8):
        r, cg = divmod(c, GRID_C)
        out[r * ML:(r + 1) * ML, cg * NL:(cg + 1) * NL] = results[c]["out"]
    return out


def _run(in_maps, **kwargs):
    from concourse.bass_utils import run_bass_kernel_spmd

    return run_bass_kernel_spmd(_get_nc(), in_maps, list(range(8)), **kwargs)


def kernel(x, y):
    res = _run(_shard(x, y))
    return _gather(res.results)


# revision 6
# speedup vs baseline: 1.1897x; 1.1897x over previous
"""Cosine-similarity matrix kernel for 8 Trainium2 NeuronCores.

Computes sim = (x_norm @ y_norm.T) / TEMP for x, y: [4096, 1024] fp32.

Strategy:
  - 4x2 grid over the output: core c = (r, cg) computes the slab
    out[r*1024:(r+1)*1024, cg*2048:(cg+1)*2048].
  - Host pre-transposes x and y so the device receives [K, M]/[K, N]
    layouts (K on partitions) and a natural-layout copy of the x shard
    for exact row-norm computation on the Scalar engine.
  - Device: raw GEMM in float32r (full-rate PE) accumulating fp32 in
    PSUM; row/col norm scales are applied during PSUM evacuation with a
    single fused DVE scalar_tensor_tensor per output tile.
  - y row-norms via a ones-vector matmul over squared tiles (partition
    reduction on the PE); x row-norms via ACT Square+accumulate on the
    natural-layout shard.
"""

import sys

sys.path.insert(0, "/opt/trn_rl_repo")

import numpy as np

TEMP = 0.05
EPS = 1e-8

N_FULL = 4096  # rows of x (output rows)
M_FULL = 4096  # rows of y (output cols)
K = 1024       # feature dim (contraction)
GRID_R = 4     # x-row shards
GRID_C = 2     # y-row shards
ML = N_FULL // GRID_R   # 1024 output rows per core
NL = M_FULL // GRID_C   # 2048 output cols per core
KT = K // 128           # 8 contraction chunks
MT = ML // 128          # 8 m-tiles per core
NS = 512                # matmul moving free dim
NH = 2                  # n-slices per psum group
NG = NL // (NS * NH)    # 2 psum groups per m-tile

_CACHE = {}


def _build():
    import concourse.bacc as bacc
    import concourse.mybir as mybir
    import concourse.tile as tile

    f32 = mybir.dt.float32
    f32r = mybir.dt.float32r
    mult = mybir.AluOpType.mult

    nc = bacc.Bacc("TRN2")
    # GEMM inputs are declared float32r (same 4-byte layout as fp32; the
    # PE's TF32 path rounds internally) so every producer in the f32r
    # dataflow writes f32r and the BIR verifier is satisfied.
    xt_d = nc.dram_tensor("xt", [K, ML], f32r, kind="ExternalInput")  # x-shard transposed [K, ML]
    xn_d = nc.dram_tensor("xn", [ML, K], f32, kind="ExternalInput")   # x-shard natural [ML, K]
    yt_d = nc.dram_tensor("yt", [K, NL], f32r, kind="ExternalInput")  # y-shard transposed [K, NL]
    ones_d = nc.dram_tensor("ones", [128, 1], f32r, kind="ExternalInput")
    out_d = nc.dram_tensor("out", [ML, NL], f32, kind="ExternalOutput")

    with tile.TileContext(nc) as tc:
        with (
            tc.tile_pool(name="big", bufs=1) as big,
            tc.tile_pool(name="sml", bufs=1) as sml,
            tc.tile_pool(name="scr", bufs=2) as scr,
            tc.tile_pool(name="ost", bufs=6) as ost,
            tc.tile_pool(name="psn", bufs=1, space="PSUM") as psn,
            tc.tile_pool(name="psg", bufs=2, space="PSUM") as psg,
        ):
            # ---- persistent SBUF inputs ----
            xt_sb = []
            yt_sb = []
            for k in range(KT):
                xt_k = big.tile([128, ML], f32r, name=f"xt{k}", tag=f"xt{k}")
                for h in range(2):
                    nc.sync.dma_start(
                        xt_k[:, h * (ML // 2):(h + 1) * (ML // 2)],
                        xt_d[k * 128:(k + 1) * 128, h * (ML // 2):(h + 1) * (ML // 2)],
                    )
                xt_sb.append(xt_k)
                yt_k = big.tile([128, NL], f32r, name=f"yt{k}", tag=f"yt{k}")
                for h in range(2):
                    nc.sync.dma_start(
                        yt_k[:, h * (NL // 2):(h + 1) * (NL // 2)],
                        yt_d[k * 128:(k + 1) * 128, h * (NL // 2):(h + 1) * (NL // 2)],
                    )
                yt_sb.append(yt_k)

            ones = sml.tile([128, 1], f32r, name="ones", tag="ones")
            nc.sync.dma_start(ones[:], ones_d[:])
            warm = sml.tile([1, 4], f32, name="warm", tag="warm")
            nc.vector.memset(warm, 1.0)
            nc.scalar.activation(warm[:, 0:1], warm[:, 1:2],
                                 mybir.ActivationFunctionType.Ln)
            nc.scalar.activation(warm[:, 2:3], warm[:, 3:4],
                                 mybir.ActivationFunctionType.Exp)

            # ---- y row norms: ones-vector matmul over squared tiles ----
            # ny2[n] = sum_k yt[k, n]^2  (partition reduction on PE, f32r)
            ny_ps = psn.tile([1, NL // NS, NS], f32, name="ny_ps", tag="norm")
            for k in range(KT):
                ysq = scr.tile([128, NL], f32r, name=f"ysq{k}", tag="ysq")
                nc.scalar.square(ysq[:], yt_sb[k][:].bitcast(f32))
                for s in range(NL // NS):
                    nc.tensor.matmul(
                        ny_ps[:, s],
                        ones[:],
                        ysq[:, s * NS:(s + 1) * NS],
                        start=(k == 0),
                        stop=(k == KT - 1),
                    )
            ry_row = sml.tile([1, NL], f32, name="ry_row", tag="ry_row")
            for s in range(NL // NS):
                nc.vector.tensor_copy(ry_row[:, s * NS:(s + 1) * NS], ny_ps[:, s])
            # ry = 1/max(sqrt(ny2), EPS) = exp(-0.5*ln(max(ny2, EPS^2)))
            # (single-lane DVE reciprocal of [1, NL] costs ~13us; ACT tables
            # give the same to ~1e-5 in two passes)
            nc.vector.tensor_scalar_max(ry_row[:], ry_row[:], EPS * EPS)
            nc.scalar.activation(ry_row[:], ry_row[:],
                                 mybir.ActivationFunctionType.Ln)
            nc.scalar.activation(ry_row[:], ry_row[:],
                                 mybir.ActivationFunctionType.Exp, scale=-0.5)
            # replicate across all 128 partitions with a K=1 fp32 matmul
            # (ones_col.T @ ry_row), then evacuate to SBUF
            ones_row = sml.tile([1, 128], f32, name="ones_row", tag="ones_row")
            nc.vector.memset(ones_row, 1.0)
            ry_ps = psn.tile([128, NL // NS, NS], f32, name="ry_ps", tag="norm")
            ry_rep = big.tile([128, NL], f32, name="ry_rep", tag="ry_rep")
            for s in range(NL // NS):
                nc.tensor.matmul(
                    ry_ps[:, s],
                    ones_row[0:1, :],
                    ry_row[0:1, s * NS:(s + 1) * NS],
                    start=True, stop=True,
                )
                nc.vector.tensor_copy(ry_rep[:, s * NS:(s + 1) * NS], ry_ps[:, s])

            # ---- x row norms from the natural-layout shard ----
            # rx[t][p] = 1 / (TEMP * max(||x_{t*128+p}||, EPS))
            rx = []
            for t in range(MT):
                xnat = scr.tile([128, K], f32, name=f"xnat{t}", tag="xnat")
                nc.sync.dma_start(xnat[:], xn_d[t * 128:(t + 1) * 128, :])
                sqd = scr.tile([128, K], f32, name=f"sqd{t}", tag="sqd")
                nx2 = sml.tile([128, 1], f32, name=f"nx2_{t}", tag=f"nx2_{t}")
                nc.scalar.activation(
                    sqd[:], xnat[:], mybir.ActivationFunctionType.Square,
                    accum_out=nx2[:],
                )
                rx_t = sml.tile([128, 1], f32, name=f"rx{t}", tag=f"rx{t}")
                nc.scalar.sqrt(rx_t[:], nx2[:])
                nc.vector.tensor_scalar_max(rx_t[:], rx_t[:], EPS)
                nc.vector.tensor_scalar_mul(rx_t[:], rx_t[:], TEMP)
                nc.vector.reciprocal(rx_t[:], rx_t[:])
                rx.append(rx_t)

            # ---- main GEMM: out[m, n] = sum_k xt[k, m] * yt[k, n] ----
            for t in range(MT):
                lhsT = lambda k: xt_sb[k][:, t * 128:(t + 1) * 128]
                for g in range(NG):
                    po = psg.tile([128, NH, NS], f32, name=f"po{t}_{g}", tag="po")
                    for k in range(KT):
                        lh = lhsT(k)
                        for s in range(NH):
                            off = (g * NH + s) * NS
                            nc.tensor.matmul(
                                po[:, s],
                                lh,
                                yt_sb[k][:, off:off + NS],
                                start=(k == 0),
                                stop=(k == KT - 1),
                            )
                    for s in range(NH):
                        off = (g * NH + s) * NS
                        osb = ost.tile([128, NS], f32, name=f"osb{t}_{g}_{s}", tag="osb")
                        nc.vector.scalar_tensor_tensor(
                            osb[:], po[:, s], rx[t][:], ry_rep[:, off:off + NS],
                            op0=mult, op1=mult,
                        )
                        nc.sync.dma_start(
                            out_d[t * 128:(t + 1) * 128, off:off + NS], osb[:]
                        )

    nc.finalize()
    return nc


def _get_nc():
    if "nc" not in _CACHE:
        _CACHE["nc"] = _build()
    return _CACHE["nc"]


def _shard(x, y):
    x = np.ascontiguousarray(np.asarray(x, dtype=np.float32))
    y = np.ascontiguousarray(np.asarray(y, dtype=np.float32))
    xT = np.ascontiguousarray(x.T)
    yT = np.ascontiguousarray(y.T)
    in_maps = []
    for c in range(8):
        r, cg = divmod(c, GRID_C)
        in_maps.append({
            "xt": np.ascontiguousarray(xT[:, r * ML:(r + 1) * ML]),
            "xn": np.ascontiguousarray(x[r * ML:(r + 1) * ML, :]),
            "yt": np.ascontiguousarray(yT[:, cg * NL:(cg + 1) * NL]),
            "ones": np.ones((128, 1), np.float32),
        })
    return in_maps


def _gather(results):
    out = np.empty((N_FULL, M_FULL), np.float32)
    for c in range(8):
        r, cg = divmod(c, GRID_C)
        out[r * ML:(r + 1) * ML, cg * NL:(cg + 1) * NL] = results[c]["out"]
    return out


def _run(in_maps, **kwargs):
    from concourse.bass_utils import run_bass_kernel_spmd

    return run_bass_kernel_spmd(_get_nc(), in_maps, list(range(8)), **kwargs)


def kernel(x, y):
    res = _run(_shard(x, y))
    return _gather(res.results)
